# revision 1
# baseline (speedup 1.0000x reference)
"""Trainium2 Bass kernel for nn_MinLSTMCell (B=8, T=4096, D=1024, H=1024).

Self-contained: hardcodes shapes/sharding. Data-parallel over batch B across
8 NeuronCores (one batch element per core), as suggested by the sharding hint.
"""


import numpy as np

import concourse.mybir as mybir
import concourse.tile as tile
from concourse import bacc

B, T, D, H = 8, 4096, 1024, 1024
TB = 512            # t-block (psum free dim)
NTB = T // TB       # 8
NHT = H // 128      # 8 h-tiles
NDK = D // 128      # 8 d-chunks
F32 = mybir.dt.float32
F32R = mybir.dt.float32r
AF = mybir.ActivationFunctionType
OP = mybir.AluOpType


def build_kernel():
    nc = bacc.Bacc()
    xt = nc.dram_tensor("xt", [D, T], F32, kind="ExternalInput")  # x transposed
    wdr = {
        g: nc.dram_tensor(f"w{g}", [D, H], F32, kind="ExternalInput")
        for g in "fih"
    }
    nbf = nc.dram_tensor("nbf", [128, NHT], F32, kind="ExternalInput")   # -bf
    hbi = nc.dram_tensor("hbi", [128, NHT], F32, kind="ExternalInput")   # bi/2
    hbh = nc.dram_tensor("hbh", [128, NHT], F32, kind="ExternalInput")   # bh/2
    b2h = nc.dram_tensor("b2h", [128, NHT], F32, kind="ExternalInput")   # 2*bh
    g4 = nc.dram_tensor("g4", [128, NHT], F32, kind="ExternalInput")     # 4*g0
    out = nc.dram_tensor("out", [H, T], F32, kind="ExternalOutput")

    with tile.TileContext(nc) as tc:
        with (
            tc.tile_pool(name="singles", bufs=1) as singles,
            tc.tile_pool(name="xtp", bufs=18) as xt_p,
            tc.tile_pool(name="pz", bufs=6, space="PSUM") as pz,
            tc.tile_pool(name="ew", bufs=3) as ew,
            tc.tile_pool(name="scan", bufs=9) as scan_p,
            tc.tile_pool(name="outp", bufs=4) as out_p,
        ):
            def emit_xload(tb):
                t0 = tb * TB
                tiles = []
                for k in range(NDK):
                    xk = xt_p.tile([128, TB], F32R, tag="xT")
                    nc.sync.dma_start(
                        xk[:],
                        xt[k * 128:(k + 1) * 128, t0:t0 + TB].bitcast(F32R),
                    )
                    tiles.append(xk)
                return tiles

            # x for tb0 loads before the weights
            xT_cur = emit_xload(0)

            # resident weights (scalar queue): per (gate, d-chunk) [128, H]
            w_sb = {}
            for g in "fih":
                for k in range(NDK):
                    t = singles.tile([128, H], F32R, tag=f"W{g}{k}")
                    eng = nc.scalar if k % 2 == 0 else nc.sync
                    eng.dma_start(
                        t[:], wdr[g][k * 128:(k + 1) * 128, :].bitcast(F32R)
                    )
                    w_sb[(g, k)] = t
            nbf_t = singles.tile([128, NHT], F32, tag="nbf")
            nc.sync.dma_start(nbf_t[:], nbf[:])
            hbi_t = singles.tile([128, NHT], F32, tag="hbi")
            nc.sync.dma_start(hbi_t[:], hbi[:])
            hbh_t = singles.tile([128, NHT], F32, tag="hbh")
            nc.sync.dma_start(hbh_t[:], hbh[:])
            b2h_t = singles.tile([128, NHT], F32, tag="b2h")
            nc.sync.dma_start(b2h_t[:], b2h[:])
            g4_t = singles.tile([128, NHT], F32, tag="g4")
            nc.sync.dma_start(g4_t[:], g4[:])

            s_prev = [None] * NHT
            for tb in range(NTB):
                t0 = tb * TB
                xT = xT_cur
                for ht in range(NHT):
                    hs = slice(ht * 128, (ht + 1) * 128)
                    z = {}
                    for g in "fih":
                        zt = pz.tile([128, TB], F32, tag="z")
                        for k in range(NDK):
                            nc.tensor.matmul(
                                zt[:],
                                w_sb[(g, k)][:, hs],
                                xT[k][:],
                                start=(k == 0),
                                stop=(k == NDK - 1),
                            )
                        z[g] = zt
                    # prefetch next block's xT
                    if tb + 1 < NTB and ht == 0:
                        xT_cur = emit_xload(tb + 1)
                    # ---- ACT phase (single table set: exp+tanh+copy+identity)
                    ef = ew.tile([128, TB], F32, tag="ef")
                    nc.scalar.activation(
                        ef[:], z["f"][:], AF.Exp,
                        bias=nbf_t[:, ht:ht + 1], scale=-1.0,
                    )
                    ti_ = ew.tile([128, TB], F32, tag="ti")
                    nc.scalar.activation(
                        ti_[:], z["i"][:], AF.Tanh,
                        bias=hbi_t[:, ht:ht + 1], scale=0.5,
                    )
                    th_ = ew.tile([128, TB], F32, tag="th")
                    nc.scalar.activation(
                        th_[:], z["h"][:], AF.Tanh,
                        bias=hbh_t[:, ht:ht + 1], scale=0.5,
                    )
                    # tip = ti + 1 (in place)
                    nc.scalar.activation(ti_[:], ti_[:], AF.Copy, bias=1.0)
                    # M = 2*zh + 2*bh
                    m_ = ew.tile([128, TB], F32, tag="m")
                    nc.scalar.activation(
                        m_[:], z["h"][:], AF.Identity,
                        bias=b2h_t[:, ht:ht + 1], scale=2.0,
                    )
                    # ---- DVE phase
                    nc.vector.tensor_tensor(m_[:], m_[:], th_[:], op=OP.max)
                    u = ew.tile([128, TB], F32, tag="u")
                    nc.vector.scalar_tensor_tensor(
                        u[:], ef[:], 1.0, ti_[:], op0=OP.add, op1=OP.mult
                    )
                    # w = (m+1)*u  (in place into m_)
                    nc.vector.scalar_tensor_tensor(
                        m_[:], m_[:], 1.0, u[:], op0=OP.add, op1=OP.mult
                    )
                    s_t = scan_p.tile([128, TB], F32, tag="S")
                    init = (
                        g4_t[:, ht:ht + 1] if tb == 0
                        else s_prev[ht][:, TB - 1:TB]
                    )
                    nc.vector.tensor_tensor_scan(
                        s_t[:], m_[:], m_[:], initial=init,
                        op0=OP.add, op1=OP.bypass,
                    )
                    s_prev[ht] = s_t
                    # dd = 2u+4 (in place), then fq = 1/dd (in place)
                    nc.scalar.activation(u[:], u[:], AF.Copy, bias=4.0, scale=2.0)
                    nc.vector.reciprocal_approx_fast(u[:], u[:])
                    o = out_p.tile([128, TB], F32, tag="o")
                    nc.vector.tensor_mul(o[:], u[:], s_t[:])
                    nc.sync.dma_start(out[hs, t0:t0 + TB], o[:])
    nc.finalize()
    return nc


_NC_CACHE = None


def get_nc():
    global _NC_CACHE
    if _NC_CACHE is None:
        _NC_CACHE = build_kernel()
    return _NC_CACHE


def kernel(x_t, h_prev, Wf, bf, Wi, bi, Wh, bh, _run_opts=None):
    from concourse.bass_utils import run_bass_kernel_spmd

    x_t = np.asarray(x_t, dtype=np.float32)
    h_prev = np.asarray(h_prev, dtype=np.float32)
    Wf = np.ascontiguousarray(np.asarray(Wf, dtype=np.float32))
    Wi = np.ascontiguousarray(np.asarray(Wi, dtype=np.float32))
    Wh = np.ascontiguousarray(np.asarray(Wh, dtype=np.float32))
    bf = np.asarray(bf, dtype=np.float32)
    bi = np.asarray(bi, dtype=np.float32)
    bh = np.asarray(bh, dtype=np.float32)

    nc = get_nc()

    g0 = np.maximum(h_prev + 0.5, 1.0 / (1.0 + np.exp(-h_prev))).astype(np.float32)
    nbf = np.ascontiguousarray((-bf).reshape(NHT, 128).T)
    hbi = np.ascontiguousarray((0.5 * bi).reshape(NHT, 128).T)
    hbh = np.ascontiguousarray((0.5 * bh).reshape(NHT, 128).T)
    b2h = np.ascontiguousarray((2.0 * bh).reshape(NHT, 128).T)

    in_maps = []
    for b in range(B):
        g4 = np.ascontiguousarray((4.0 * g0[b]).reshape(NHT, 128).T)
        in_maps.append({
            "xt": np.ascontiguousarray(x_t[b].T),
            "wf": Wf, "wi": Wi, "wh": Wh,
            "nbf": nbf, "hbi": hbi, "hbh": hbh, "b2h": b2h,
            "g4": g4,
        })

    opts = _run_opts or {}
    res = run_bass_kernel_spmd(nc, in_maps, core_ids=list(range(B)), **opts)

    out = np.empty((B, T + 1, H), dtype=np.float32)
    for b in range(B):
        out[b, 0, :] = g0[b]
        out[b, 1:, :] = res.results[b]["out"].T
    if _run_opts is not None:
        return out, res
    return out



# revision 14
# speedup vs baseline: 1.6430x; 1.6430x over previous
"""Trainium2 Bass kernel for nn_MinLSTMCell (B=8, T=4096, D=1024, H=1024).

Self-contained: hardcodes shapes/sharding. Data-parallel over batch B across
8 NeuronCores (one batch element per core).

Math (equivalent to the reference's cumsum-in-exp-space form):
  ef  = exp(-(zf+bf));  u2 = 2u = (1+ef)*(2+2*tanh((zi+bi)/2))
  g   = max(zh+bh+0.5, sigmoid(zh+bh));  w = g*u2
  S_t = 4*g0 + cumsum(w);  h_t = S_t / (u2_t + 4)

Precision plan (gate is absmax-normalized 2e-2; simulated err ~1.31e-2):
  - zh GEMM fully in fp8e4m3 DoubleRow perf mode (half-cost); x and Wh each
    scaled by 16 to stay out of fp8 subnormals -> psum zh = 256*(x@Wh)
  - zf, zi GEMMs: first 256 of D in one fp8-DR pass (unscaled x, 16*W), the
    remaining 768 in fp16 with weights pre-scaled by 16 (exact in fp16)
    -> psum = 16*(x@W); the ACT scale folds the 1/16 (or 1/256) back out
  - elementwise fp16; scan state fp32 internally; dd/fq fp32; output fp16
Engine placement: ACT does the psum-reading ops (per-partition bias is free)
plus dd; DVE does the rest 1024-wide (two t-blocks paired) so 2x/4x 16-bit
perf modes and per-op overheads amortize.
"""


import numpy as np

import concourse.mybir as mybir
import concourse.tile as tile
from concourse import bacc

B, T, D, H = 8, 4096, 1024, 1024
TB = 512            # t-block (psum free dim)
TP = 2 * TB         # paired t-block for 1024-wide DVE ops
NTP = T // TP       # 4 pairs
NHT = H // 128      # 8 h-tiles
NK16 = 6            # fp16 k-subtiles (d = 256..1023) for f/i
NP8 = D // 256      # 4 DoubleRow passes (fp8 path, zh)
SC = 16.0
F32 = mybir.dt.float32
F16 = mybir.dt.float16
FP8 = mybir.dt.float8e4
AF = mybir.ActivationFunctionType
OP = mybir.AluOpType
DR = mybir.MatmulPerfMode.DoubleRow


def build_kernel():
    nc = bacc.Bacc()
    x16 = nc.dram_tensor("x16", [128, NK16, T], F16, kind="ExternalInput")
    x8 = nc.dram_tensor("x8", [128, 8, T], FP8, kind="ExternalInput")    # 16*x
    x8b = nc.dram_tensor("x8b", [128, 2, T], FP8, kind="ExternalInput")  # x[:256]
    wf = nc.dram_tensor("wf", [128, NK16, H], F16, kind="ExternalInput")  # 16*Wf[256:]
    wi = nc.dram_tensor("wi", [128, NK16, H], F16, kind="ExternalInput")
    wf8 = nc.dram_tensor("wf8", [128, 2, H], FP8, kind="ExternalInput")  # 16*Wf[:256]
    wi8 = nc.dram_tensor("wi8", [128, 2, H], FP8, kind="ExternalInput")
    wh = nc.dram_tensor("wh", [128, 8, H], FP8, kind="ExternalInput")    # 16*Wh
    nbf = nc.dram_tensor("nbf", [128, NHT], F32, kind="ExternalInput")  # -bf
    hbi = nc.dram_tensor("hbi", [128, NHT], F32, kind="ExternalInput")  # bi/2
    hbh = nc.dram_tensor("hbh", [128, NHT], F32, kind="ExternalInput")  # bh/2
    bg = nc.dram_tensor("bg", [128, NHT], F32, kind="ExternalInput")    # bh+0.5
    g4 = nc.dram_tensor("g4", [128, NHT], F32, kind="ExternalInput")    # 4*g0
    out = nc.dram_tensor("out", [H, T], F16, kind="ExternalOutput")

    with tile.TileContext(nc) as tc:
        with (
            tc.tile_pool(name="singles", bufs=1) as singles,
            tc.tile_pool(name="x16p", bufs=12) as x16_p,
            tc.tile_pool(name="x8p", bufs=8) as x8_p,
            tc.tile_pool(name="x8bp", bufs=2) as x8b_p,
            tc.tile_pool(name="pz", bufs=8, space="PSUM") as pz,
            tc.tile_pool(name="ewb", bufs=4) as ewb,
            tc.tile_pool(name="ewf", bufs=2) as ewf,
            tc.tile_pool(name="scan", bufs=9) as scan_p,
            tc.tile_pool(name="outp", bufs=3) as out_p,
        ):
            def emit_xload(tp):
                t0 = tp * TP
                xs16 = []
                for k in range(NK16):
                    xk = x16_p.tile([128, TP], F16, tag="x16")
                    nc.sync.dma_start(xk[:], x16[:, k, t0:t0 + TP])
                    xs16.append(xk)
                xs8 = []
                for p in range(NP8):
                    xp = x8_p.tile([128, 2, TP], FP8, tag="x8")
                    nc.sync.dma_start(xp[:], x8[:, 2 * p:2 * p + 2, t0:t0 + TP])
                    xs8.append(xp)
                xb = x8b_p.tile([128, 2, TP], FP8, tag="x8b")
                nc.sync.dma_start(xb[:], x8b[:, :, t0:t0 + TP])
                return xs16, xs8, xb

            x_cur = emit_xload(0)

            # resident weights (split per k-subtile for early start)
            wf_sb, wi_sb = [], []
            for k in range(NK16):
                t = singles.tile([128, H], F16, tag=f"wf{k}")
                eng = nc.scalar if k % 2 == 0 else nc.sync
                eng.dma_start(t[:], wf[:, k, :])
                wf_sb.append(t)
            for k in range(NK16):
                t = singles.tile([128, H], F16, tag=f"wi{k}")
                eng = nc.scalar if k % 2 == 0 else nc.sync
                eng.dma_start(t[:], wi[:, k, :])
                wi_sb.append(t)
            wf8_sb = singles.tile([128, 2, H], FP8, tag="wf8")
            nc.scalar.dma_start(wf8_sb[:], wf8[:])
            wi8_sb = singles.tile([128, 2, H], FP8, tag="wi8")
            nc.scalar.dma_start(wi8_sb[:], wi8[:])
            wh_sb = []
            for p in range(NP8):
                t = singles.tile([128, 2, H], FP8, tag=f"wh{p}")
                eng = nc.scalar if p % 2 == 0 else nc.sync
                eng.dma_start(t[:], wh[:, 2 * p:2 * p + 2, :])
                wh_sb.append(t)
            nbf_t = singles.tile([128, NHT], F32, tag="nbf")
            nc.sync.dma_start(nbf_t[:], nbf[:])
            hbi_t = singles.tile([128, NHT], F32, tag="hbi")
            nc.sync.dma_start(hbi_t[:], hbi[:])
            hbh_t = singles.tile([128, NHT], F32, tag="hbh")
            nc.sync.dma_start(hbh_t[:], hbh[:])
            bg_t = singles.tile([128, NHT], F32, tag="bg")
            nc.sync.dma_start(bg_t[:], bg[:])
            g4_t = singles.tile([128, NHT], F32, tag="g4")
            nc.sync.dma_start(g4_t[:], g4[:])

            s_prev = [None] * NHT
            for tp in range(NTP):
                t0 = tp * TP
                xs16, xs8, xb = x_cur
                for ht in range(NHT):
                    hs = slice(ht * 128, (ht + 1) * 128)
                    ef = ewb.tile([128, TP], F16, tag="ef")
                    ti2 = ewb.tile([128, TP], F16, tag="ti2")
                    th = ewb.tile([128, TP], F16, tag="th")
                    pg = ewb.tile([128, TP], F16, tag="pg")
                    zf0 = pz.tile([128, TB], F32, tag="z")
                    zf1 = pz.tile([128, TB], F32, tag="z")
                    cs0, cs1 = slice(0, TB), slice(TB, TP)
                    nc.tensor.matmul(zf0[:], wf8_sb[:, :, hs], xb[:, :, cs0],
                                     start=True, stop=False, perf_mode=DR)
                    nc.tensor.matmul(zf1[:], wf8_sb[:, :, hs], xb[:, :, cs1],
                                     start=True, stop=False, perf_mode=DR)
                    for k in range(NK16):
                        st = k == NK16 - 1
                        nc.tensor.matmul(zf0[:], wf_sb[k][:, hs],
                                         xs16[k][:, cs0], start=False, stop=st)
                        nc.tensor.matmul(zf1[:], wf_sb[k][:, hs],
                                         xs16[k][:, cs1], start=False, stop=st)
                    nc.scalar.activation(
                        ef[:, cs0], zf0[:], AF.Exp,
                        bias=nbf_t[:, ht:ht + 1], scale=-1.0 / 16.0)
                    nc.scalar.activation(
                        ef[:, cs1], zf1[:], AF.Exp,
                        bias=nbf_t[:, ht:ht + 1], scale=-1.0 / 16.0)
                    zi0 = pz.tile([128, TB], F32, tag="z")
                    zi1 = pz.tile([128, TB], F32, tag="z")
                    nc.tensor.matmul(zi0[:], wi8_sb[:, :, hs], xb[:, :, cs0],
                                     start=True, stop=False, perf_mode=DR)
                    nc.tensor.matmul(zi1[:], wi8_sb[:, :, hs], xb[:, :, cs1],
                                     start=True, stop=False, perf_mode=DR)
                    for k in range(NK16):
                        st = k == NK16 - 1
                        nc.tensor.matmul(zi0[:], wi_sb[k][:, hs],
                                         xs16[k][:, cs0], start=False, stop=st)
                        nc.tensor.matmul(zi1[:], wi_sb[k][:, hs],
                                         xs16[k][:, cs1], start=False, stop=st)
                    nc.scalar.activation(
                        ti2[:, cs0], zi0[:], AF.Tanh,
                        bias=hbi_t[:, ht:ht + 1], scale=1.0 / 32.0)
                    nc.scalar.activation(
                        ti2[:, cs1], zi1[:], AF.Tanh,
                        bias=hbi_t[:, ht:ht + 1], scale=1.0 / 32.0)
                    zh0 = pz.tile([128, TB], F32, tag="z")
                    zh1 = pz.tile([128, TB], F32, tag="z")
                    for p in range(NP8):
                        sa, st = p == 0, p == NP8 - 1
                        nc.tensor.matmul(zh0[:], wh_sb[p][:, :, hs],
                                         xs8[p][:, :, cs0], start=sa, stop=st,
                                         perf_mode=DR)
                        nc.tensor.matmul(zh1[:], wh_sb[p][:, :, hs],
                                         xs8[p][:, :, cs1], start=sa, stop=st,
                                         perf_mode=DR)
                    if tp + 1 < NTP and ht == 0:
                        x_cur = emit_xload(tp + 1)
                    for half, zh in ((0, zh0), (1, zh1)):
                        cs = cs0 if half == 0 else cs1
                        nc.scalar.activation(
                            th[:, cs], zh[:], AF.Tanh,
                            bias=hbh_t[:, ht:ht + 1], scale=1.0 / 512.0)
                        nc.scalar.activation(
                            pg[:, cs], zh[:], AF.Identity,
                            bias=bg_t[:, ht:ht + 1], scale=1.0 / 256.0)

                    # ---- pair-wide (1024) elementwise
                    tip2 = ewb.tile([128, TP], F16, tag="tip2")
                    nc.vector.tensor_scalar(
                        tip2[:], ti2[:], 2.0, 2.0, op0=OP.mult, op1=OP.add
                    )
                    sg = ewb.tile([128, TP], F16, tag="sg")
                    nc.vector.tensor_scalar(
                        sg[:], th[:], 0.5, 0.5, op0=OP.mult, op1=OP.add
                    )
                    u2 = ewb.tile([128, TP], F16, tag="u2")
                    nc.vector.scalar_tensor_tensor(
                        u2[:], ef[:], 1.0, tip2[:], op0=OP.add, op1=OP.mult
                    )
                    g_ = ewb.tile([128, TP], F16, tag="g")
                    nc.vector.tensor_tensor(g_[:], pg[:], sg[:], op=OP.max)
                    w_ = ewb.tile([128, TP], F16, tag="w")
                    nc.vector.tensor_tensor(w_[:], g_[:], u2[:], op=OP.mult)
                    dd = ewf.tile([128, TP], F32, tag="dd")
                    nc.scalar.activation(
                        dd[:], u2[:], AF.Copy, bias=4.0, scale=1.0
                    )
                    # scan: fp16 in/out, fp32 internal state. data1 is a
                    # same-shape dummy (op1=bypass ignores it); pointing it
                    # at a different tile avoids dual-fetch of one address.
                    s_t = scan_p.tile([128, TP], F16, tag="S")
                    init = (
                        g4_t[:, ht:ht + 1] if tp == 0
                        else s_prev[ht][:, TP - 1:TP]
                    )
                    nc.vector.tensor_tensor_scan(
                        s_t[:], w_[:], u2[:], initial=init,
                        op0=OP.add, op1=OP.bypass,
                    )
                    s_prev[ht] = s_t
                    fq = ewf.tile([128, TP], F32, tag="fq")
                    nc.vector.reciprocal_approx_fast(fq[:], dd[:])
                    o = out_p.tile([128, TP], F16, tag="o")
                    nc.vector.tensor_tensor(o[:], s_t[:], fq[:], op=OP.mult)
                    nc.scalar.dma_start(out[hs, t0:t0 + TP], o[:])
    nc.finalize()
    return nc


_NC_CACHE = None


def get_nc():
    global _NC_CACHE
    if _NC_CACHE is None:
        _NC_CACHE = build_kernel()
    return _NC_CACHE


def _rearr(a, nk):
    # [nk*128, N] -> [128, nk, N] with row d = kt*128 + p  ->  [p, kt, :]
    return np.ascontiguousarray(
        a.reshape(nk, 128, a.shape[1]).transpose(1, 0, 2)
    )


def kernel(x_t, h_prev, Wf, bf, Wi, bi, Wh, bh, _run_opts=None):
    import ml_dtypes
    from concourse.bass_utils import run_bass_kernel_spmd

    E4 = ml_dtypes.float8_e4m3

    x_t = np.asarray(x_t, dtype=np.float32)
    h_prev = np.asarray(h_prev, dtype=np.float32)
    Wf = np.asarray(Wf, dtype=np.float32)
    Wi = np.asarray(Wi, dtype=np.float32)
    Wh = np.asarray(Wh, dtype=np.float32)
    bf = np.asarray(bf, dtype=np.float32)
    bi = np.asarray(bi, dtype=np.float32)
    bh = np.asarray(bh, dtype=np.float32)

    nc = get_nc()

    g0 = np.maximum(h_prev + 0.5, 1.0 / (1.0 + np.exp(-h_prev))).astype(np.float32)
    wf16 = _rearr(SC * Wf[256:], NK16).astype(np.float16)
    wi16 = _rearr(SC * Wi[256:], NK16).astype(np.float16)
    wf8 = _rearr(SC * Wf[:256], 2).astype(E4)
    wi8 = _rearr(SC * Wi[:256], 2).astype(E4)
    wh8 = _rearr(SC * Wh, 8).astype(E4)
    nbf = np.ascontiguousarray((-bf).reshape(NHT, 128).T)
    hbi = np.ascontiguousarray((0.5 * bi).reshape(NHT, 128).T)
    hbh = np.ascontiguousarray((0.5 * bh).reshape(NHT, 128).T)
    bg = np.ascontiguousarray((bh + 0.5).reshape(NHT, 128).T)

    in_maps = []
    for b in range(B):
        xT = x_t[b].T  # [D, T]
        g4 = np.ascontiguousarray((4.0 * g0[b]).reshape(NHT, 128).T)
        in_maps.append({
            "x16": _rearr(xT[256:], NK16).astype(np.float16),
            "x8": _rearr(SC * xT, 8).astype(E4),
            "x8b": _rearr(xT[:256], 2).astype(E4),
            "wf": wf16, "wi": wi16, "wf8": wf8, "wi8": wi8, "wh": wh8,
            "nbf": nbf, "hbi": hbi, "hbh": hbh, "bg": bg,
            "g4": g4,
        })

    opts = _run_opts or {}
    res = run_bass_kernel_spmd(nc, in_maps, core_ids=list(range(B)), **opts)

    out = np.empty((B, T + 1, H), dtype=np.float32)
    for b in range(B):
        out[b, 0, :] = g0[b]
        out[b, 1:, :] = res.results[b]["out"].astype(np.float32).T
    if _run_opts is not None:
        return out, res
    return out
